# revision 6
# baseline (speedup 1.0000x reference)
"""MoE feed-forward (sin-gated, top-2 routing) on 8 Trainium2 NeuronCores.

Strategy: expert-parallel. The router (a [T,8] matmul + softmax + top-2,
~0.05% of total FLOPs) runs on the host as part of dispatch; tokens are
gathered per expert, padded to a uniform capacity C, and each core runs the
dense FFN for one expert in bf16 (fp32 accumulation). The host combines the
per-expert outputs with the top-2 gate weights.

Self-contained: hardcodes B=2, S=2048, D=1024, H=2816, E=8, TOP_K=2.
"""

import numpy as np

try:
    import concourse.bass as bass
except ImportError:  # pragma: no cover
    import sys

    sys.path.insert(0, "/opt/trn_rl_repo")
    import concourse.bass as bass

import bass_rust
import concourse.tile as tile
import ml_dtypes
from concourse import mybir
from concourse.bass_utils import run_bass_kernel_spmd
from concourse.vector_clock import ScopedClock

BF16 = ml_dtypes.bfloat16

# The walrus CoreV3 codegen in this container rejects instructions carrying
# more than one semaphore wait ("Too many sync wait commands"). Post-process
# the instruction stream: move excess waits onto same-engine nops inserted
# immediately before the offending instruction.
_MAX_INST_WAITS = 1


def _split_multi_waits(nc: bass.Bass, limit: int = _MAX_INST_WAITS):
    blocks = list(nc.m.functions[0].blocks)
    snapshots = {bb.name: list(bb.instructions) for bb in blocks}
    inserts: dict[str, list] = {}
    for insts in snapshots.values():
        for inst in insts:
            si = inst.sync_info
            waits = list(si.on_wait) if (si is not None and si.on_wait) else []
            if len(waits) <= limit:
                continue
            si.on_wait = waits[-limit:]
            nops = []
            for w in waits[:-limit]:
                nop = nc.engines[inst.engine].nop().ins
                nop.sync_info = bass_rust.SyncInfo(on_wait=[w], on_update=[])
                nops.append(nop)
            inserts[inst.name] = nops
    if not inserts:
        return
    for bb in blocks:
        out = []
        for inst in snapshots[bb.name]:
            out.extend(inserts.get(inst.name, ()))
            out.append(inst)
        bb.instructions = out

D = 1024
H = 2816
E = 8
TOP_K = 2
N_CORES = 8
NK = D // 128  # 8  contraction chunks for matmul 1
NH = H // 128  # 22 h-chunks
ND = D // 128  # 8  output chunks for matmul 2
EPS_NORM = 1e-12

_nc_cache: dict = {}
LAST_RESULT = None  # BassKernelResults of the most recent run (for profiling)
TRACE = False


def _ct_tiles(C: int) -> list[tuple[int, int]]:
    """Split [0, C) into matmul free-dim tiles (multiples of 128, <= 512)."""
    assert C % 128 == 0
    nct = -(-C // 512)
    base = (C // nct) // 128 * 128
    sizes = [base] * nct
    leftover = (C - base * nct) // 128
    for i in range(leftover):
        sizes[i] += 128
    out, c0 = [], 0
    for s in sizes:
        out.append((c0, s))
        c0 += s
    assert c0 == C
    return out


def _build(C: int) -> bass.Bass:
    """One expert's FFN: yT = w2^T @ (sin(w1^T xT) * (w3^T xT)), all [.,C]."""
    cts = _ct_tiles(C)
    nc = bass.Bass()
    xT = nc.declare_dram_parameter("xT", [NK, 128, C], mybir.dt.bfloat16, isOutput=False)
    w1 = nc.declare_dram_parameter("w1", [NH, 128, D], mybir.dt.bfloat16, isOutput=False)
    w3 = nc.declare_dram_parameter("w3", [NH, 128, D], mybir.dt.bfloat16, isOutput=False)
    w2 = nc.declare_dram_parameter("w2", [NH, 128, D], mybir.dt.bfloat16, isOutput=False)
    yT = nc.declare_dram_parameter("yT", [ND, 128, C], mybir.dt.float32, isOutput=True)
    SIN = mybir.ActivationFunctionType.Sin

    with tile.TileContext(nc) as tc:
        with (
            tc.tile_pool(name="xpool", bufs=1) as xpool,
            tc.tile_pool(name="ypool", bufs=ND) as ypool,
            tc.tile_pool(name="wpool", bufs=3) as wpool,
            tc.tile_pool(name="w2pool", bufs=4) as w2pool,
            tc.tile_pool(name="hpool", bufs=4) as hpool,
            tc.tile_pool(name="spool", bufs=3) as spool,
            tc.tile_pool(name="abpool", bufs=2, space="PSUM") as abpool,
            tc.tile_pool(name="opool", bufs=2, space="PSUM") as opool,
        ):
            # Resident activations: x^T as 8 chunks of [128, C] on partitions.
            xts = xpool.tile([128, NK, C], mybir.dt.bfloat16)
            for k in range(NK):
                nc.sync.dma_start(out=xts[:, k, :], in_=xT[k])

            # fp32 output accumulators, one [128, C] per 128-wide d-chunk.
            yas = []
            for d in range(ND):
                ya = ypool.tile([128, C], mybir.dt.float32, tag="ya", name=f"ya{d}")
                yas.append(ya)

            # March over h in pairs: matmul-2 accumulates both h-chunks of a
            # pair in PSUM before one DVE add into the fp32 accumulator.
            for pr in range(NH // 2):
                h0 = 2 * pr
                hs = []
                for h in (h0, h0 + 1):
                    w1h = wpool.tile([128, D], mybir.dt.bfloat16, tag="w1h", name=f"w1h_{h}")
                    nc.sync.dma_start(out=w1h, in_=w1[h])
                    w3h = wpool.tile([128, D], mybir.dt.bfloat16, tag="w3h", name=f"w3h_{h}")
                    nc.sync.dma_start(out=w3h, in_=w3[h])
                    hsb = hpool.tile([128, C], mybir.dt.bfloat16, tag="hsb", name=f"hsb_{h}")
                    for c0, cn in cts:
                        pa = abpool.tile([128, 512], mybir.dt.float32, tag="pa", name=f"pa_{h}_{c0}")
                        pb = abpool.tile([128, 512], mybir.dt.float32, tag="pb", name=f"pb_{h}_{c0}")
                        for k in range(NK):
                            nc.tensor.matmul(
                                pa[:, :cn],
                                w1h[:, k * 128 : (k + 1) * 128],
                                xts[:, k, c0 : c0 + cn],
                                start=(k == 0),
                                stop=(k == NK - 1),
                            )
                        for k in range(NK):
                            nc.tensor.matmul(
                                pb[:, :cn],
                                w3h[:, k * 128 : (k + 1) * 128],
                                xts[:, k, c0 : c0 + cn],
                                start=(k == 0),
                                stop=(k == NK - 1),
                            )
                        ssb = spool.tile([128, 512], mybir.dt.bfloat16, tag="ssb", name=f"ssb_{h}_{c0}")
                        nc.scalar.activation(out=ssb[:, :cn], in_=pa[:, :cn], func=SIN)
                        nc.vector.tensor_mul(hsb[:, c0 : c0 + cn], ssb[:, :cn], pb[:, :cn])
                    hs.append(hsb)

                w2a = w2pool.tile([128, D], mybir.dt.bfloat16, tag="w2h", name=f"w2h_{h0}")
                nc.sync.dma_start(out=w2a, in_=w2[h0])
                w2b = w2pool.tile([128, D], mybir.dt.bfloat16, tag="w2h", name=f"w2h_{h0 + 1}")
                nc.sync.dma_start(out=w2b, in_=w2[h0 + 1])
                for c0, cn in cts:
                    for d in range(ND):
                        po = opool.tile([128, 512], mybir.dt.float32, tag="po", name=f"po_{pr}_{c0}_{d}")
                        nc.tensor.matmul(
                            po[:, :cn],
                            w2a[:, d * 128 : (d + 1) * 128],
                            hs[0][:, c0 : c0 + cn],
                            start=True,
                            stop=False,
                        )
                        nc.tensor.matmul(
                            po[:, :cn],
                            w2b[:, d * 128 : (d + 1) * 128],
                            hs[1][:, c0 : c0 + cn],
                            start=False,
                            stop=True,
                        )
                        if pr == 0:
                            nc.vector.tensor_copy(yas[d][:, c0 : c0 + cn], po[:, :cn])
                        else:
                            nc.vector.tensor_add(
                                yas[d][:, c0 : c0 + cn],
                                yas[d][:, c0 : c0 + cn],
                                po[:, :cn],
                            )

            for d in range(ND):
                nc.sync.dma_start(out=yT[d], in_=yas[d])

    _split_multi_waits(nc)
    return nc


def _route(x, router_w, router_b):
    """Replicates the reference router in fp32 numpy."""
    B, S, _ = x.shape
    T = B * S
    xf = x.reshape(T, D)
    logits = (xf @ router_w).reshape(B, S, E) + router_b
    nrm = np.maximum(np.sqrt((logits * logits).sum(axis=1, keepdims=True)), EPS_NORM)
    ln = (logits / nrm).astype(np.float32)
    m = ln.max(axis=-1, keepdims=True)
    ex = np.exp(ln - m)
    probs = ex / ex.sum(axis=-1, keepdims=True)
    aux = np.float32((((1.0 / E) - probs.mean(axis=0)) ** 2).sum())
    pf = probs.reshape(T, E)
    idx = np.argpartition(-pf, TOP_K - 1, axis=-1)[:, :TOP_K]
    wts = np.take_along_axis(pf, idx, axis=-1)
    return idx, wts, aux


def prepare(x, w1, w2, w3, router_w, router_b):
    """Host-side routing + dispatch: returns (nc, in_maps, sels, gates, aux, C)."""
    x = np.ascontiguousarray(np.asarray(x, dtype=np.float32))
    w1 = np.asarray(w1, dtype=np.float32)
    w2 = np.asarray(w2, dtype=np.float32)
    w3 = np.asarray(w3, dtype=np.float32)
    router_w = np.asarray(router_w, dtype=np.float32)
    router_b = np.asarray(router_b, dtype=np.float32)
    B, S, _ = x.shape
    T = B * S
    xf = x.reshape(T, D)

    idx, wts, aux = _route(x, router_w, router_b)

    sels, gates = [], []
    for e in range(E):
        mask = idx == e
        sel = np.nonzero(mask.any(axis=-1))[0]
        sels.append(sel)
        gates.append((wts * mask).sum(axis=-1)[sel].astype(np.float32))

    max_cnt = max(len(s) for s in sels)
    C = max(128, -(-max_cnt // 128) * 128)

    if C not in _nc_cache:
        _nc_cache[C] = _build(C)
    nc = _nc_cache[C]

    in_maps = []
    for e in range(E):
        sel = sels[e]
        xTe = np.zeros((D, C), dtype=BF16)
        xTe[:, : len(sel)] = xf[sel].T.astype(BF16)
        w1_r = np.ascontiguousarray(
            w1[e].astype(BF16).reshape(NK, 128, NH, 128).transpose(2, 1, 0, 3)
        ).reshape(NH, 128, D)
        w3_r = np.ascontiguousarray(
            w3[e].astype(BF16).reshape(NK, 128, NH, 128).transpose(2, 1, 0, 3)
        ).reshape(NH, 128, D)
        w2_r = np.ascontiguousarray(w2[e].astype(BF16)).reshape(NH, 128, D)
        in_maps.append(
            {
                "xT": xTe.reshape(NK, 128, C),
                "w1": w1_r,
                "w3": w3_r,
                "w2": w2_r,
            }
        )
    return nc, in_maps, sels, gates, aux, C


def combine(results, sels, gates, C, B, S):
    T = B * S
    out = np.zeros((T, D), dtype=np.float32)
    for e in range(E):
        sel = sels[e]
        ye = results[e]["yT"].reshape(D, C)
        out[sel] += gates[e][:, None] * ye[:, : len(sel)].T
    return out.reshape(B, S, D)


def kernel(x, w1, w2, w3, router_w, router_b):
    global LAST_RESULT
    B, S, _ = np.asarray(x).shape
    nc, in_maps, sels, gates, aux, C = prepare(x, w1, w2, w3, router_w, router_b)
    LAST_RESULT = run_bass_kernel_spmd(nc, in_maps, list(range(N_CORES)), trace=TRACE)
    out = combine(LAST_RESULT.results, sels, gates, C, B, S)
    return out, aux


# revision 10
# speedup vs baseline: 23.7000x; 23.7000x over previous
"""MoE feed-forward (sin-gated, top-2 routing) on 8 Trainium2 NeuronCores.

Strategy: expert-parallel. The router (a [T,8] matmul + softmax + top-2,
~0.05% of total FLOPs) runs on the host as part of dispatch; tokens are
gathered per expert, padded to a uniform capacity C, and each core runs the
dense FFN for one expert in bf16 (fp32 accumulation). The host combines the
per-expert outputs with the top-2 gate weights.

Self-contained: hardcodes B=2, S=2048, D=1024, H=2816, E=8, TOP_K=2.
"""

import numpy as np

try:
    import concourse.bass as bass
except ImportError:  # pragma: no cover
    import sys

    sys.path.insert(0, "/opt/trn_rl_repo")
    import concourse.bass as bass

import bass_rust
import concourse.tile as tile
import ml_dtypes
from concourse import mybir
from concourse.bass_utils import run_bass_kernel_spmd
from concourse.vector_clock import ScopedClock

BF16 = ml_dtypes.bfloat16

# The walrus CoreV3 codegen in this container rejects instructions carrying
# more than one semaphore wait ("Too many sync wait commands"). Post-process
# the instruction stream: move excess waits onto same-engine nops inserted
# immediately before the offending instruction.
_MAX_INST_WAITS = 1


def _split_multi_waits(nc: bass.Bass, limit: int = _MAX_INST_WAITS):
    blocks = list(nc.m.functions[0].blocks)
    snapshots = {bb.name: list(bb.instructions) for bb in blocks}
    inserts: dict[str, list] = {}
    for insts in snapshots.values():
        for inst in insts:
            si = inst.sync_info
            waits = list(si.on_wait) if (si is not None and si.on_wait) else []
            if len(waits) <= limit:
                continue
            si.on_wait = waits[-limit:]
            nops = []
            for w in waits[:-limit]:
                nop = nc.engines[inst.engine].nop().ins
                nop.sync_info = bass_rust.SyncInfo(on_wait=[w], on_update=[])
                nops.append(nop)
            inserts[inst.name] = nops
    if not inserts:
        return
    for bb in blocks:
        out = []
        for inst in snapshots[bb.name]:
            out.extend(inserts.get(inst.name, ()))
            out.append(inst)
        bb.instructions = out

D = 1024
H = 2816
E = 8
TOP_K = 2
N_CORES = 8
NK = D // 128  # 8  contraction chunks for matmul 1
NH = H // 128  # 22 h-chunks
ND = D // 128  # 8  output chunks for matmul 2
EPS_NORM = 1e-12

_nc_cache: dict = {}
LAST_RESULT = None  # BassKernelResults of the most recent run (for profiling)
TRACE = False


def _ct_tiles(C: int) -> list[tuple[int, int]]:
    """Split [0, C) into matmul free-dim tiles (multiples of 128, <= 512)."""
    assert C % 128 == 0
    nct = -(-C // 512)
    base = (C // nct) // 128 * 128
    sizes = [base] * nct
    leftover = (C - base * nct) // 128
    for i in range(leftover):
        sizes[i] += 128
    out, c0 = [], 0
    for s in sizes:
        out.append((c0, s))
        c0 += s
    assert c0 == C
    return out


def _build(C: int, repeat: int = 1, kct_swap: bool = False) -> bass.Bass:
    """One expert's FFN: yT = w2^T @ (sin(w1^T xT) * (w3^T xT)), all [.,C].

    Phase A: h_sb[h] = sin(w1h^T x^T) * (w3h^T x^T) for all 22 h-chunks,
    streaming w1/w3 chunks; h_sb and w2 stay SBUF-resident.
    Phase B: for each 128-wide d-chunk, accumulate all 22 h contributions in
    a single PSUM chain, copy once to SBUF (on ACT), DMA out.
    `repeat` re-runs the compute body for differential wall-clock timing.
    """
    cts = _ct_tiles(C)
    nc = bass.Bass()
    xT = nc.declare_dram_parameter("xT", [NK, 128, C], mybir.dt.bfloat16, isOutput=False)
    w1 = nc.declare_dram_parameter("w1", [NH, 128, D], mybir.dt.bfloat16, isOutput=False)
    w3 = nc.declare_dram_parameter("w3", [NH, 128, D], mybir.dt.bfloat16, isOutput=False)
    w2 = nc.declare_dram_parameter("w2", [NH, 128, D], mybir.dt.bfloat16, isOutput=False)
    yT = nc.declare_dram_parameter("yT", [ND, 128, C], mybir.dt.float32, isOutput=True)
    SIN = mybir.ActivationFunctionType.Sin

    with tile.TileContext(nc) as tc:
        with (
            tc.tile_pool(name="xpool", bufs=1) as xpool,
            tc.tile_pool(name="w2rpool", bufs=1) as w2rpool,
            tc.tile_pool(name="hrpool", bufs=1) as hrpool,
            tc.tile_pool(name="wpool", bufs=3) as wpool,
            tc.tile_pool(name="spool", bufs=3) as spool,
            tc.tile_pool(name="ostpool", bufs=2) as ostpool,
            tc.tile_pool(name="abpool", bufs=(1 if kct_swap else 2), space="PSUM") as abpool,
            tc.tile_pool(name="opool", bufs=(2 if kct_swap else 3), space="PSUM") as opool,
        ):
            # Residents: x^T chunks, all of w2, and the gated hidden h^T.
            xts = xpool.tile([128, NK, C], mybir.dt.bfloat16)
            for k in range(NK):
                nc.scalar.dma_start(out=xts[:, k, :], in_=xT[k])
            w2sb = w2rpool.tile([128, NH, D], mybir.dt.bfloat16)
            for h in range(NH):
                nc.scalar.dma_start(out=w2sb[:, h, :], in_=w2[h])
            hsb = hrpool.tile([128, NH, C], mybir.dt.bfloat16)

            for _rep in range(repeat):
                # ---- Phase A: hidden activations, streaming w1/w3 ----
                for h in range(NH):
                    w1h = wpool.tile([128, D], mybir.dt.bfloat16, tag="w1h", name=f"w1h_{h}")
                    nc.sync.dma_start(out=w1h, in_=w1[h])
                    w3h = wpool.tile([128, D], mybir.dt.bfloat16, tag="w3h", name=f"w3h_{h}")
                    nc.sync.dma_start(out=w3h, in_=w3[h])
                    if kct_swap:
                        # Stationary-weight reuse: one LDWEIGHTS feeds all C
                        # tiles. Needs 2*len(cts) PSUM banks live.
                        pas = [
                            abpool.tile([128, 512], mybir.dt.float32, tag=f"pa{i}", name=f"pa{i}_{h}")
                            for i in range(len(cts))
                        ]
                        pbs = [
                            abpool.tile([128, 512], mybir.dt.float32, tag=f"pb{i}", name=f"pb{i}_{h}")
                            for i in range(len(cts))
                        ]
                        for wh, ps in ((w1h, pas), (w3h, pbs)):
                            for k in range(NK):
                                for i, (c0, cn) in enumerate(cts):
                                    nc.tensor.matmul(
                                        ps[i][:, :cn],
                                        wh[:, k * 128 : (k + 1) * 128],
                                        xts[:, k, c0 : c0 + cn],
                                        start=(k == 0),
                                        stop=(k == NK - 1),
                                    )
                        for i, (c0, cn) in enumerate(cts):
                            ssb = spool.tile([128, 512], mybir.dt.bfloat16, tag="ssb", name=f"ssb_{h}_{c0}")
                            nc.scalar.activation(out=ssb[:, :cn], in_=pas[i][:, :cn], func=SIN)
                            nc.vector.tensor_mul(hsb[:, h, c0 : c0 + cn], ssb[:, :cn], pbs[i][:, :cn])
                        continue
                    for c0, cn in cts:
                        pa = abpool.tile([128, 512], mybir.dt.float32, tag="pa", name=f"pa_{h}_{c0}")
                        pb = abpool.tile([128, 512], mybir.dt.float32, tag="pb", name=f"pb_{h}_{c0}")
                        for k in range(NK):
                            nc.tensor.matmul(
                                pa[:, :cn],
                                w1h[:, k * 128 : (k + 1) * 128],
                                xts[:, k, c0 : c0 + cn],
                                start=(k == 0),
                                stop=(k == NK - 1),
                            )
                        for k in range(NK):
                            nc.tensor.matmul(
                                pb[:, :cn],
                                w3h[:, k * 128 : (k + 1) * 128],
                                xts[:, k, c0 : c0 + cn],
                                start=(k == 0),
                                stop=(k == NK - 1),
                            )
                        ssb = spool.tile([128, 512], mybir.dt.bfloat16, tag="ssb", name=f"ssb_{h}_{c0}")
                        nc.scalar.activation(out=ssb[:, :cn], in_=pa[:, :cn], func=SIN)
                        nc.vector.tensor_mul(hsb[:, h, c0 : c0 + cn], ssb[:, :cn], pb[:, :cn])

                # ---- Phase B: yT[d] = sum_h w2[h,d]^T @ h_sb[h] ----
                for d in range(ND):
                    ost = ostpool.tile([128, C], mybir.dt.float32, tag="ost", name=f"ost_{d}")
                    for c0, cn in cts:
                        po = opool.tile([128, 512], mybir.dt.float32, tag="po", name=f"po_{d}_{c0}")
                        for h in range(NH):
                            nc.tensor.matmul(
                                po[:, :cn],
                                w2sb[:, h, d * 128 : (d + 1) * 128],
                                hsb[:, h, c0 : c0 + cn],
                                start=(h == 0),
                                stop=(h == NH - 1),
                            )
                        nc.scalar.copy(ost[:, c0 : c0 + cn], po[:, :cn])
                    nc.sync.dma_start(out=yT[d], in_=ost)

    _split_multi_waits(nc)
    return nc


def _route(x, router_w, router_b):
    """Replicates the reference router in fp32 numpy."""
    B, S, _ = x.shape
    T = B * S
    xf = x.reshape(T, D)
    logits = (xf @ router_w).reshape(B, S, E) + router_b
    nrm = np.maximum(np.sqrt((logits * logits).sum(axis=1, keepdims=True)), EPS_NORM)
    ln = (logits / nrm).astype(np.float32)
    m = ln.max(axis=-1, keepdims=True)
    ex = np.exp(ln - m)
    probs = ex / ex.sum(axis=-1, keepdims=True)
    aux = np.float32((((1.0 / E) - probs.mean(axis=0)) ** 2).sum())
    pf = probs.reshape(T, E)
    idx = np.argpartition(-pf, TOP_K - 1, axis=-1)[:, :TOP_K]
    wts = np.take_along_axis(pf, idx, axis=-1)
    return idx, wts, aux


def prepare(x, w1, w2, w3, router_w, router_b):
    """Host-side routing + dispatch: returns (nc, in_maps, sels, gates, aux, C)."""
    x = np.ascontiguousarray(np.asarray(x, dtype=np.float32))
    w1 = np.asarray(w1, dtype=np.float32)
    w2 = np.asarray(w2, dtype=np.float32)
    w3 = np.asarray(w3, dtype=np.float32)
    router_w = np.asarray(router_w, dtype=np.float32)
    router_b = np.asarray(router_b, dtype=np.float32)
    B, S, _ = x.shape
    T = B * S
    xf = x.reshape(T, D)

    idx, wts, aux = _route(x, router_w, router_b)

    sels, gates = [], []
    for e in range(E):
        mask = idx == e
        sel = np.nonzero(mask.any(axis=-1))[0]
        sels.append(sel)
        gates.append((wts * mask).sum(axis=-1)[sel].astype(np.float32))

    max_cnt = max(len(s) for s in sels)
    C = max(128, -(-max_cnt // 128) * 128)

    if C not in _nc_cache:
        _nc_cache[C] = _build(C)
    nc = _nc_cache[C]

    in_maps = []
    for e in range(E):
        sel = sels[e]
        xTe = np.zeros((D, C), dtype=BF16)
        xTe[:, : len(sel)] = xf[sel].T.astype(BF16)
        w1_r = np.ascontiguousarray(
            w1[e].astype(BF16).reshape(NK, 128, NH, 128).transpose(2, 1, 0, 3)
        ).reshape(NH, 128, D)
        w3_r = np.ascontiguousarray(
            w3[e].astype(BF16).reshape(NK, 128, NH, 128).transpose(2, 1, 0, 3)
        ).reshape(NH, 128, D)
        w2_r = np.ascontiguousarray(w2[e].astype(BF16)).reshape(NH, 128, D)
        in_maps.append(
            {
                "xT": xTe.reshape(NK, 128, C),
                "w1": w1_r,
                "w3": w3_r,
                "w2": w2_r,
            }
        )
    return nc, in_maps, sels, gates, aux, C


def combine(results, sels, gates, C, B, S):
    T = B * S
    out = np.zeros((T, D), dtype=np.float32)
    for e in range(E):
        sel = sels[e]
        ye = results[e]["yT"].reshape(D, C)
        out[sel] += gates[e][:, None] * ye[:, : len(sel)].T
    return out.reshape(B, S, D)


def kernel(x, w1, w2, w3, router_w, router_b):
    global LAST_RESULT
    B, S, _ = np.asarray(x).shape
    nc, in_maps, sels, gates, aux, C = prepare(x, w1, w2, w3, router_w, router_b)
    LAST_RESULT = run_bass_kernel_spmd(nc, in_maps, list(range(N_CORES)), trace=TRACE)
    out = combine(LAST_RESULT.results, sels, gates, C, B, S)
    return out, aux
